# revision 35
# baseline (speedup 1.0000x reference)
"""DifferentialAttention (B=2, S=2048, D=2048, H=16, KVH=8) on 8 TRN2 NeuronCores.

Sharding: 8 cores = 2 (batch) x 4 (tensor-parallel head groups).
Core c = 4*b + r handles batch b and real heads 4r..4r+3.

v2 architecture (scores in [q, k] layout):
  - projections in bf16 (resident weights), RoPE on DVE
  - scores S[q-part, k-free] per 128-q tile; causal diag masked by adding
    -1e30 on the mixed 128x128 block (Pool engine, pre-exp)
  - exp on ACT with accum_out -> softmax row-sums r1, r2 for free
  - D = alpha*E2 - E1 (alpha = lam*r1/r2 per-q partition scalar, one DVE
    stt per q-tile); RMS-norm scale-invariance absorbs the 1/r1 and the
    sign (folded into Wo on host)
  - D transposed to [k, q] via DMA-engine xbar transpose (bf16), then a
    SINGLE AV matmul per k-tile (vs 2 + 2 rowsum matmuls before)
  - var/eps: var = ones @ D^2 + onecol @ (128*eps*r1^2-row); rsqrt via
    exp(-0.5*ln(x)); otf = U * sf in bf16
  - o_proj bf16, psum->bf16 copies split DVE/Pool, bf16 output summed on host
"""

import math
import numpy as np

B, S, D = 2, 2048, 2048
H, KVH = 16, 8
Dh = 64
TP = 4
NCORES = 8
LAYER_IDX = 2
LAMBDA_INIT = 0.8 - 0.6 * math.exp(-0.3 * LAYER_IDX)
EPS = 1e-5
ROPE_THETA = 10000.0

_CACHE = {}


def _build_nc():
    import concourse.bass as bass  # noqa: F401
    import concourse.tile as tile
    from concourse import bacc, mybir

    F32 = mybir.dt.float32
    BF16 = mybir.dt.bfloat16
    Act = mybir.ActivationFunctionType
    Alu = mybir.AluOpType

    nc = bacc.Bacc("TRN2", target_bir_lowering=False, debug=False)

    xT = nc.dram_tensor("xT", [D, S], BF16, kind="ExternalInput")
    wqT = nc.dram_tensor("wqT", [D, 512], BF16, kind="ExternalInput")
    wkT = nc.dram_tensor("wkT", [D, 256], BF16, kind="ExternalInput")
    wvT = nc.dram_tensor("wvT", [D, 256], BF16, kind="ExternalInput")
    woT = nc.dram_tensor("woT", [512, D], BF16, kind="ExternalInput")
    cosT_d = nc.dram_tensor("cosT", [128, S], BF16, kind="ExternalInput")
    ssinT_d = nc.dram_tensor("ssinT", [128, S], BF16, kind="ExternalInput")
    trineg_d = nc.dram_tensor("trineg", [128, 128], F32, kind="ExternalInput")
    ones_d = nc.dram_tensor("ones", [128, 128], BF16, kind="ExternalInput")
    onecol_d = nc.dram_tensor("onecol", [1, 128], BF16, kind="ExternalInput")
    lam_d = nc.dram_tensor("lam", [128, 1], F32, kind="ExternalInput")
    out_d = nc.dram_tensor("out", [S, D], BF16, kind="ExternalOutput")
    # scratch for transposing r1^2 from [q-part, 1] to a [1, q-free] row
    rr_d = nc.dram_tensor("rr", [16, 128, 4], BF16, kind="Internal")

    KD = 16  # contraction tiles of 128 over D=2048
    SEPS = 128.0 * EPS  # varp/128 = mean(u^2) + eps*r1^2

    with tile.TileContext(nc) as tc:
        with tc.tile_pool(name="const", bufs=1) as constp, \
             tc.tile_pool(name="persist", bufs=1) as persist:

            cosT = constp.tile([128, S], BF16, tag="cos")
            ssinT = constp.tile([128, S], BF16, tag="ssin")
            trineg = constp.tile([128, 128], F32, tag="trineg")
            ones = constp.tile([128, 128], BF16, tag="ones")
            onecol = constp.tile([1, 128], BF16, tag="onecol")
            lam = constp.tile([128, 1], F32, tag="lam")

            qT_sb = [persist.tile([128, S], BF16, tag=f"qT{m}", name=f"qT{m}")
                     for m in range(4)]
            kT_sb = [persist.tile([128, S], BF16, tag=f"kT{m}", name=f"kT{m}")
                     for m in range(4)]
            v_sb = [persist.tile([128, 256], BF16, tag=f"v{ms}", name=f"v{ms}")
                    for ms in range(16)]
            otf = [persist.tile([128, S], BF16, tag=f"otf{p}", name=f"otf{p}")
                   for p in range(4)]
            # resident weights
            wq_r = [persist.tile([128, 2048], BF16, tag=f"wqr{kp}", name=f"wqr{kp}")
                    for kp in range(4)]
            wk_r = [persist.tile([128, 2048], BF16, tag=f"wkr{kp}", name=f"wkr{kp}")
                    for kp in range(2)]
            wv_r = [persist.tile([128, 2048], BF16, tag=f"wvr{kp}", name=f"wvr{kp}")
                    for kp in range(2)]
            wo_r = [persist.tile([128, 2048], BF16, tag=f"wor{n}", name=f"wor{n}")
                    for n in range(4)]

            # ---- Unified phase A (projections) + phase B (attention) ----
            # A shards are emitted INSIDE the B block stream so the
            # ACT-bound exp pipeline overlaps the PE-bound projections.
            with tc.tile_pool(name="xtp", bufs=4) as xtp, \
                 tc.tile_pool(name="ropet", bufs=2) as rp, \
                 tc.tile_pool(name="esb", bufs=3) as esb, \
                 tc.tile_pool(name="dsb", bufs=2) as dsb, \
                 tc.tile_pool(name="dtb", bufs=2) as dtb, \
                 tc.tile_pool(name="rsb", bufs=14) as rsb, \
                 tc.tile_pool(name="ebp", bufs=2) as ebp, \
                 tc.tile_pool(name="prp", bufs=1) as prp, \
                 tc.tile_pool(name="outp", bufs=2) as outp, \
                 tc.tile_pool(name="psA", bufs=2, space="PSUM") as psA, \
                 tc.tile_pool(name="psS", bufs=2, space="PSUM") as psS, \
                 tc.tile_pool(name="psUVC", bufs=2, space="PSUM") as psUVC:

                pre_cat = prp.tile([128, 2048], F32, tag="pre", name="pre_cat")
                state = {}
                a_xt = {}

                def load_w(dst, src, nblk):
                    nc.sync.dma_start(
                        out=dst[:].rearrange("p (b n) -> p b n", b=nblk),
                        in_=src.rearrange("(b p) n -> p b n", b=nblk),
                    )

                def rope_to(ps, dst_slice, c0):
                    gsl = slice(c0, c0 + 512)
                    qraw = rp.tile([128, 512], BF16, tag="qraw", name="qraw")
                    nc.vector.tensor_copy(qraw[:], ps[:])
                    qsw = rp.tile([128, 512], BF16, tag="qsw", name="qsw")
                    for blk in range(4):
                        sb_ = (blk ^ 1) * 32
                        nc.gpsimd.tensor_copy(
                            qsw[blk * 32:blk * 32 + 32, :], qraw[sb_:sb_ + 32, :])
                    nc.vector.tensor_mul(qraw[:], qraw[:], cosT[:, gsl])
                    nc.gpsimd.tensor_mul(qsw[:], qsw[:], ssinT[:, gsl])
                    nc.vector.tensor_add(dst_slice, qraw[:], qsw[:])

                def xt_rhs(sh, kd):
                    return a_xt[sh][kd // 4][:, (kd % 4) * 512:(kd % 4) * 512 + 512]

                def emit_A(sh, part):
                    c0 = 512 * sh
                    if part == 0:
                        # x tiles + q projection for m in {0, 1}
                        xt4 = []
                        for kp in range(4):
                            t = xtp.tile([128, 2048], BF16, tag="xt",
                                         name=f"xt{kp}")
                            nsplit = 4 if (sh == 0 and kp == 0) else 1
                            step = 4 // nsplit
                            for s in range(nsplit):
                                nc.sync.dma_start(
                                    out=t[:, s * step * 512:(s + 1) * step * 512]
                                        .rearrange("p (b n) -> p b n", b=step),
                                    in_=xT[kp * 512 + s * step * 128:
                                           kp * 512 + (s + 1) * step * 128,
                                           c0:c0 + 512]
                                        .rearrange("(b p) n -> p b n", b=step),
                                )
                                if sh == 0 and kp == 0:
                                    load_w(wq_r[0][:, s * 512:(s + 1) * 512],
                                           wqT[s * 128:(s + 1) * 128, :], 1)
                            if sh == 0 and kp > 0:
                                load_w(wq_r[kp],
                                       wqT[kp * 512:kp * 512 + 512, :], 4)
                            if sh == 0 and kp == 1:
                                nc.sync.dma_start(out=cosT[:], in_=cosT_d[:])
                                nc.sync.dma_start(out=ssinT[:], in_=ssinT_d[:])
                            if sh == 0 and kp == 2:
                                for kk in range(2):
                                    load_w(wk_r[kk],
                                           wkT[kk * 1024:kk * 1024 + 1024, :], 8)
                            if sh == 0 and kp == 3:
                                for kk in range(2):
                                    load_w(wv_r[kk],
                                           wvT[kk * 1024:kk * 1024 + 1024, :], 8)
                                nc.sync.dma_start(out=trineg[:], in_=trineg_d[:])
                                nc.sync.dma_start(out=ones[:], in_=ones_d[:])
                                nc.sync.dma_start(out=onecol[:], in_=onecol_d[:])
                                nc.sync.dma_start(out=lam[:], in_=lam_d[:])
                            xt4.append(t)
                        a_xt[sh] = xt4
                    if part in (0, 1):
                        # q projection, two heads per pass (2 psum banks)
                        qps = {}
                        for kp in range(4):
                            for t in range(4):
                                kd = kp * 4 + t
                                for m in (0, 1) if part == 0 else (2, 3):
                                    if kd == 0:
                                        qps[m] = psA.tile(
                                            [128, 512], F32, tag="pa",
                                            name=f"qps{m}")
                                    nc.tensor.matmul(
                                        qps[m][:],
                                        wq_r[kp][:, t * 512 + m * 128:
                                                 t * 512 + m * 128 + 128],
                                        xt_rhs(sh, kd),
                                        start=(kd == 0), stop=(kd == KD - 1),
                                    )
                        for m in qps:
                            rope_to(qps[m], qT_sb[m][:, c0:c0 + 512], c0)
                    if part == 2:
                        # k projection + RoPE + duplicate halves
                        kps = {}
                        for kd in range(KD):
                            for m in range(2):
                                if kd == 0:
                                    kps[m] = psA.tile([128, 512], F32, tag="pa",
                                                      name=f"kps{m}")
                                nc.tensor.matmul(
                                    kps[m][:],
                                    wk_r[kd // 8][:, (kd % 8) * 256 + m * 128:
                                                  (kd % 8) * 256 + m * 128 + 128],
                                    xt_rhs(sh, kd),
                                    start=(kd == 0), stop=(kd == KD - 1),
                                )
                        for m in range(2):
                            ktmp = rp.tile([128, 512], BF16, tag="ktmp",
                                           name="ktmp")
                            rope_to(kps[m], ktmp[:], c0)
                            for e in range(2):
                                src_ = ktmp[e * 64:e * 64 + 64, :]
                                for hh in range(2):
                                    nc.gpsimd.tensor_copy(
                                        kT_sb[2 * m + e][hh * 64:hh * 64 + 64,
                                                         c0:c0 + 512], src_)
                    if part == 3:
                        # v projection, two s-tiles per pass
                        for half in range(2):
                            vps = {}
                            for kd in range(KD):
                                for ms in (0, 1) if half == 0 else (2, 3):
                                    if kd == 0:
                                        vps[ms] = psA.tile(
                                            [128, 512], F32, tag="pa",
                                            name=f"vps{ms}")
                                    nc.tensor.matmul(
                                        vps[ms][:, 0:256],
                                        xt_rhs(sh, kd)[:, ms * 128:ms * 128 + 128],
                                        wv_r[kd // 8][:, (kd % 8) * 256:
                                                      (kd % 8) * 256 + 256],
                                        start=(kd == 0), stop=(kd == KD - 1),
                                    )
                            for ms in vps:
                                nc.vector.tensor_copy(v_sb[sh * 4 + ms][:],
                                                      vps[ms][:, 0:256])
                        del a_xt[sh]
                        if sh == 1:
                            for n in range(4):
                                load_w(wo_r[n], woT[:, n * 512:n * 512 + 512], 4)

                def stage1(p, qb, hook=None):
                    mk = p // 2
                    DT = dtb.tile([128, 16, 4, 128], BF16, tag="dt", name="DT")
                    r1c = rsb.tile([128, 4], F32, tag="r", name="r1c")
                    r2c = rsb.tile([128, 4], F32, tag="r", name="r2c")
                    rx1 = rsb.tile([128, 4], F32, tag="r", name="rx1")
                    rx2 = rsb.tile([128, 4], F32, tag="r", name="rx2")
                    rcp = rsb.tile([128, 4], F32, tag="r", name="rcp")
                    alc = rsb.tile([128, 4], F32, tag="r", name="alc")
                    r1s = rsb.tile([128, 4], BF16, tag="r", name="r1s")
                    for qi_ in range(4):
                        qt = 4 * qb + qi_
                        W = (qt + 1) * 128
                        q0 = qt * 128
                        two = W > 1024
                        E = {}
                        for half in range(2):
                            e_t = esb.tile([128, 2048], BF16, tag="e",
                                           name=f"E{half}")
                            E[half] = e_t
                            co = 0
                            ci = 0
                            while co < W:
                                cw = min(1024, W - co)
                                sps = psS.tile([128, 1024], F32, tag="s",
                                               name="sps")
                                s0 = 0
                                while s0 < cw:
                                    sw = min(512, cw - s0)
                                    nc.tensor.matmul(
                                        sps[:, s0:s0 + sw],
                                        qT_sb[p][64 * half:64 * half + 64,
                                                 q0:q0 + 128],
                                        kT_sb[p][64 * half:64 * half + 64,
                                                 co + s0:co + s0 + sw],
                                        start=True, stop=True,
                                    )
                                    s0 += sw
                                if co + cw == W:
                                    # diag 128x128 block: causal mask pre-exp
                                    dsl = slice(cw - 128, cw)
                                    nc.vector.tensor_add(
                                        sps[:, dsl], sps[:, dsl], trineg[:])
                                acc = (r1c if half == 0 else r2c) if ci == 0 \
                                    else (rx1 if half == 0 else rx2)
                                nc.scalar.activation(
                                    e_t[:, co:co + cw], sps[:, 0:cw], Act.Exp,
                                    accum_out=acc[:, qi_:qi_ + 1])
                                co += cw
                                ci += 1
                        # per-qt softmax-sum scalars (column qi_)
                        qsl = slice(qi_, qi_ + 1)
                        if two:
                            nc.vector.tensor_add(r1c[:, qsl], r1c[:, qsl],
                                                 rx1[:, qsl])
                            nc.vector.tensor_add(r2c[:, qsl], r2c[:, qsl],
                                                 rx2[:, qsl])
                        nc.vector.reciprocal(rcp[:, qsl], r2c[:, qsl])
                        nc.vector.scalar_tensor_tensor(
                            alc[:, qsl], r1c[:, qsl], lam[:, 0:1], rcp[:, qsl],
                            Alu.mult, Alu.mult)
                        # D-combine + transpose, chunked so the transpose
                        # starts while later exps still run
                        d_t = dsb.tile([128, 2048], BF16, tag="d", name="Dt")
                        co = 0
                        while co < W:
                            cw = min(1024, W - co)
                            nc.vector.scalar_tensor_tensor(
                                d_t[:, co:co + cw], E[1][:, co:co + cw],
                                alc[:, qsl], E[0][:, co:co + cw],
                                Alu.mult, Alu.subtract)
                            nc.sync.dma_start_transpose(
                                DT[:, co // 128:(co + cw) // 128, qi_:qi_ + 1, :],
                                d_t[:, co:co + cw])
                            co += cw
                        if hook is not None:
                            hook(qi_)
                    # eps row: r1s = 128*eps*r1^2, bounced via DRAM to a row
                    nc.vector.scalar_tensor_tensor(
                        r1s[:], r1c[:], SEPS, r1c[:], Alu.mult, Alu.mult)
                    r1row = rsb.tile([1, 512], BF16, tag="rrow", name="r1row",
                                     bufs=3)
                    blk = p * 4 + qb
                    nc.sync.dma_start(out=rr_d[blk], in_=r1s[:])
                    nc.sync.dma_start(
                        out=r1row[:].rearrange("o (q p) -> o q p", q=4),
                        in_=rr_d[blk].rearrange("p q -> () q p"))
                    return DT, r1row

                def av_part(p, qb, DT, U, part):
                    # quarter `part` of the AV accumulation (off-diag first)
                    mk = p // 2
                    nj = 4 * qb + 4
                    order = list(range(4 * qb)) + list(range(4 * qb, nj))
                    first = order[0]
                    npart = (nj + 3) // 4
                    for j in order[part * npart:(part + 1) * npart]:
                        t_ = j - 4 * qb
                        vc = t_ * 128 if t_ > 0 else 0
                        nc.tensor.matmul(
                            U[:, vc:512],
                            v_sb[j][:, mk * 128:mk * 128 + 128],
                            DT[:, j:j + 1, vc // 128:4, :],
                            start=(j == first), stop=(j == nj - 1),
                        )

                def var_tail(p, qb, r1row, U):
                    # u -> otf (bf16); var = ones @ u^2 + onecol @ eps*r1^2
                    osl = otf[p][:, qb * 512:qb * 512 + 512]
                    nc.vector.tensor_copy(osl, U[:])
                    sq = ebp.tile([128, 512], BF16, tag="eb", name="sq")
                    nc.gpsimd.tensor_mul(sq[:], osl, osl)
                    varp = psUVC.tile([128, 512], F32, tag="uvc", name="varp")
                    nc.tensor.matmul(varp[:], ones[:], sq[:],
                                     start=True, stop=False)
                    nc.tensor.matmul(varp[:], onecol[:], r1row[:],
                                     start=False, stop=True)
                    nc.vector.tensor_copy(pre_cat[:, p * 512:p * 512 + 512], varp[:])

                def qb_epilogue(qb):
                    # sf = (pre/128)^-0.5 via ln+exp, batched per qb row
                    nc.scalar.activation(pre_cat[:], pre_cat[:], Act.Ln,
                                         scale=1.0 / 128.0)
                    nc.scalar.activation(pre_cat[:], pre_cat[:], Act.Exp,
                                         scale=-0.5)
                    for p in range(4):
                        nc.vector.tensor_mul(
                            otf[p][:, qb * 512:qb * 512 + 512],
                            otf[p][:, qb * 512:qb * 512 + 512],
                            pre_cat[:, p * 512:p * 512 + 512])

                def emit_C_tile(m):
                    # o_proj for one 128-row seq tile
                    if True:
                        for n in range(4):
                            ps = psUVC.tile([128, 512], F32, tag="uvc", name="pc")
                            for kc in range(4):
                                nc.tensor.matmul(
                                    ps[:],
                                    otf[kc][:, m * 128:m * 128 + 128],
                                    wo_r[n][:, kc * 512:kc * 512 + 512],
                                    start=(kc == 0), stop=(kc == 3),
                                )
                            osb = outp.tile([128, 512], BF16, tag="ob", name="osb")
                            nc.vector.tensor_copy(osb[:], ps[:])
                            nc.sync.dma_start(
                                out=out_d[m * 128:m * 128 + 128,
                                          n * 512:n * 512 + 512],
                                in_=osb[:])

                for sh_ in (0, 1):
                    for part_ in range(4):
                        emit_A(sh_, part_)
                rows = [1, 2, 3, 0]
                blocks = [(qb, p) for qb in rows for p in range(4)]
                c_queue = []
                for i, (qb, p) in enumerate(blocks):
                    if i >= 1:
                        qm, pm = blocks[i - 1]
                        DTm, r1m = state.pop(i - 1)
                        Um = psUVC.tile([128, 512], F32, tag="uvc", name="U")

                        def hook(qi_, qm=qm, pm=pm, DTm=DTm, Um=Um):
                            av_part(pm, qm, DTm, Um, qi_)
                    else:
                        hook = None
                    state[i] = stage1(p, qb, hook)
                    if i >= 1:
                        var_tail(pm, qm, r1m, Um)
                        if pm == 3:
                            qb_epilogue(qm)
                            c_queue.extend(range(4 * qm, 4 * qm + 4))
                        if c_queue and i >= 5:
                            emit_C_tile(c_queue.pop(0))
                            if len(c_queue) >= 5:
                                emit_C_tile(c_queue.pop(0))
                    nsh = {1: 2, 2: 3}.get(qb)
                    if nsh is not None:
                        emit_A(nsh, p)
                qm, pm = blocks[-1]
                DTm, r1m = state.pop(len(blocks) - 1)
                Um = psUVC.tile([128, 512], F32, tag="uvc", name="U")
                for g in range(4):
                    av_part(pm, qm, DTm, Um, g)
                var_tail(pm, qm, r1m, Um)
                qb_epilogue(qm)
                c_queue.extend(range(0, 4))
                for mm in c_queue:
                    emit_C_tile(mm)

    nc.compile()
    return nc


def _host_tables():
    import ml_dtypes
    bf16 = ml_dtypes.bfloat16
    inv = ROPE_THETA ** (-np.arange(Dh, dtype=np.float64) / Dh)
    pos = np.arange(S, dtype=np.float64)
    fr = pos[:, None] * inv[None, :]              # [S, 64]
    cos = np.cos(fr).astype(np.float32)           # [S, 64]
    sin = np.sin(fr).astype(np.float32)
    d = np.arange(128) % 64
    cosT = cos[:, d].T.astype(bf16)               # [128, S]
    sgn = np.where((np.arange(128) % 64) < 32, -1.0, 1.0).astype(np.float32)
    ssinT = (sin[:, d].T * sgn[:, None]).astype(bf16)
    # trineg[q, k] = 0 if k <= q else -1e30 (strictly upper)
    trineg = np.where(np.arange(128)[None, :] > np.arange(128)[:, None],
                      np.float32(-1e30), np.float32(0.0))
    ones = np.ones((128, 128), bf16)
    onecol = np.ones((1, 128), bf16)
    return (np.ascontiguousarray(cosT), np.ascontiguousarray(ssinT),
            np.ascontiguousarray(trineg), ones, onecol)


def kernel(hidden_states, Wq, Wk, Wv, Wo,
           lambda_q1, lambda_k1, lambda_q2, lambda_k2, subln_weight):
    import ml_dtypes
    from concourse.bass_utils import run_bass_kernel_spmd

    bf16 = ml_dtypes.bfloat16
    if "nc" not in _CACHE:
        _CACHE["nc"] = _build_nc()
        _CACHE["tables"] = _host_tables()
    nc = _CACHE["nc"]
    cosT, ssinT, trineg, ones, onecol = _CACHE["tables"]

    f32 = np.float32
    hs = np.asarray(hidden_states, f32)
    Wq = np.asarray(Wq, f32)
    Wk = np.asarray(Wk, f32)
    Wv = np.asarray(Wv, f32)
    Wo = np.asarray(Wo, f32)
    subln = np.asarray(subln_weight, f32)

    lam1 = np.exp(np.sum(np.asarray(lambda_q1, f32) * np.asarray(lambda_k1, f32),
                         dtype=f32))
    lam2 = np.exp(np.sum(np.asarray(lambda_q2, f32) * np.asarray(lambda_k2, f32),
                         dtype=f32))
    lam_full = f32(lam1 - lam2 + LAMBDA_INIT)
    lam_arr = np.full((128, 1), lam_full, f32)

    scale = f32(Dh ** -0.5)
    # sign flip folded here: device computes -u
    wprime = (np.tile(subln, H) * f32(-(1.0 - LAMBDA_INIT))).astype(f32)
    WoS = Wo * wprime[None, :]

    in_maps = []
    for c in range(NCORES):
        b, r = c // TP, c % TP
        in_maps.append({
            "xT": np.ascontiguousarray(hs[b].T).astype(bf16),
            "wqT": np.ascontiguousarray((Wq[512 * r:512 * r + 512, :] * scale).T)
                .astype(bf16),
            "wkT": np.ascontiguousarray(Wk[256 * r:256 * r + 256, :].T).astype(bf16),
            "wvT": np.ascontiguousarray(Wv[256 * r:256 * r + 256, :].T).astype(bf16),
            "woT": np.ascontiguousarray(WoS[:, 512 * r:512 * r + 512].T).astype(bf16),
            "cosT": cosT, "ssinT": ssinT, "trineg": trineg, "ones": ones,
            "onecol": onecol, "lam": lam_arr,
        })

    res = run_bass_kernel_spmd(nc, in_maps, core_ids=list(range(NCORES)))
    out = np.zeros((B, S, D), f32)
    for c in range(NCORES):
        out[c // TP] += np.asarray(res.results[c]["out"], f32)
    return out
